# revision 14
# baseline (speedup 1.0000x reference)
"""CA3 recurrent-matrix retrieval kernel for 8 Trainium2 NeuronCores.

Reference computation (see problem): 5 steps of
    sim       = cosine_similarity(current, memory)          # [B, C]
    weights   = softmax(sim, axis=-1)
    attracted = weights @ memory                            # [B, D]
    current   = 0.8 * attracted + 0.2 * current
with B=256, C=100000, D=1024.

Distribution: memory_matrix is sharded row-wise (capacity dim C) across the
8 cores.  Because cosine similarity is bounded in [-1, 1], softmax needs no
max subtraction: each core accumulates the unnormalized readout
U = sum_c exp(sim_c) * mem[c] and S = sum_c exp(sim_c) over its shard, a
single AllReduce combines (U, S) across cores, and every core redundantly
applies current = 0.8 * U/S + 0.2 * current.

Per-core per-step sweep over 98 chunks of 128 memory rows:
  - simT[cs,b] accumulated over 8 K-chunks of D (PE), memory supplied in a
    host-pretransposed tiled layout so no on-device transposes are needed
  - expT = exp(inv_mem_norm[cs] * simT + bias) fused on ACT (bias=-30 kills
    the 352 zero-padding rows)
  - U[b,d] and S[b] accumulated in PSUM across all 98 chunks (PE)

Matmul inputs are float32r (same fp32 bytes; the PE rounds inputs RNE at 11
mantissa bits, measured) which streams at 1 cycle/row vs 4 for fp32.
"""

import numpy as np

import concourse.bass as bass
import concourse.tile as tile
from concourse import bacc, mybir
from concourse import bass_utils
from concourse.masks import make_identity

B, C, D = 256, 100000, 1024
NCORES = 8
CSP = 12544                 # per-core shard rows (C padded to 8*12544=100352)
NCHUNK = CSP // 128         # 98
STEPS = 5
EPS = 1e-8
PAD_BIAS = -30.0            # exp(-30) ~ 1e-13: padding rows vanish
FP = mybir.dt.float32
F32R = mybir.dt.float32r    # same fp32 bytes, single-pass reduced multiply:
                            # 1 cycle/row on PE (vs 4 for fp32) when N>=256
BF16 = mybir.dt.bfloat16
AOP = mybir.AluOpType
AFT = mybir.ActivationFunctionType

# dtype of the two streamed memory copies + matmul operands:
#   F32R: full fp32 traffic, PE rounds inputs to 11 mantissa bits
#   BF16: half the HBM traffic, 8 mantissa bits
MM_DT = F32R


def _emit(nc, nchunk=NCHUNK, steps=STEPS, reps=1, mm_dt=None):
    if mm_dt is None:
        mm_dt = MM_DT
    csp = nchunk * 128
    memT = nc.dram_tensor("memT", [csp, D], mm_dt, kind="ExternalInput").ap()
    memN = nc.dram_tensor("memN", [csp, D + 16], mm_dt, kind="ExternalInput").ap()
    scl = nc.dram_tensor("scl", [128, nchunk], FP, kind="ExternalInput").ap()
    bia = nc.dram_tensor("bia", [128, nchunk], FP, kind="ExternalInput").ap()
    qry = nc.dram_tensor("qry", [B, D], FP, kind="ExternalInput").ap()
    out = nc.dram_tensor("out", [B, D], FP, kind="ExternalOutput").ap()

    with tile.TileContext(nc) as tc:
        with (
            tc.tile_pool(name="singles", bufs=1) as singles,
            tc.tile_pool(name="state", bufs=1) as state,
            tc.tile_pool(name="mh", bufs=8) as mh_pool,
            tc.tile_pool(name="mn", bufs=8) as mn_pool,
            tc.tile_pool(name="expp", bufs=4) as exp_pool,
            tc.tile_pool(name="scr", bufs=2) as scr_pool,
            tc.tile_pool(name="smallps", bufs=2, space="PSUM") as smallps,
            tc.tile_pool(name="ups", bufs=1, space="PSUM") as u_pool,
            tc.tile_pool(name="seps", bufs=1, space="PSUM") as se_pool,
            tc.tile_pool(name="dram", bufs=2, space="DRAM") as dram_pool,
        ):
            pools = dict(scr=scr_pool, sps=smallps, ups=u_pool, seps=se_pool,
                         dram=dram_pool, mh=mh_pool, mn=mn_pool, expp=exp_pool)
            ident = singles.tile([128, 128], FP)
            make_identity(nc, ident[:])
            scl_sb = singles.tile([128, nchunk], FP)
            nc.sync.dma_start(out=scl_sb, in_=scl)
            bia_sb = singles.tile([128, nchunk], FP)
            nc.sync.dma_start(out=bia_sb, in_=bia)

            # persistent iteration state: current in [B, D] layout, 2 B-tiles
            cur = [state.tile([128, D], FP, name=f"cur{bt}") for bt in range(2)]
            # chatT: normalized current, transposed: [d-part, dchunk, b]
            chatT = state.tile([128, 8, 256], mm_dt)

            for rep in range(reps):
                for bt in range(2):
                    nc.sync.dma_start(
                        out=cur[bt], in_=qry[bt * 128:(bt + 1) * 128, :])
                for step in range(steps):
                    _emit_step(nc, pools, cur, chatT, ident,
                               scl_sb, bia_sb, memT, memN, nchunk,
                               f"r{rep}s{step}", mm_dt)

            for bt in range(2):
                nc.sync.dma_start(out=out[bt * 128:(bt + 1) * 128, :],
                                  in_=cur[bt])

    return nc


def _emit_step(nc, pools, cur, chatT, ident, scl_sb, bia_sb,
               memT, memN, nchunk, sid, mm_dt):
    scr_pool = pools["scr"]
    smallps = pools["sps"]

    # ---- row norms of current, inv-norm, chat = current/||current||
    ss = scr_pool.tile([128, 2], FP, name=f"ss_{sid}", tag="ss")
    chat = []
    for bt in range(2):
        sq = scr_pool.tile([128, D], FP, name=f"sq_{sid}_{bt}", tag="sq")
        nc.vector.tensor_mul(sq, cur[bt], cur[bt])
        nc.vector.reduce_sum(ss[:, bt:bt + 1], sq, axis=mybir.AxisListType.X)
    nrm = scr_pool.tile([128, 2], FP, name=f"nrm_{sid}", tag="nrm")
    nc.scalar.sqrt(nrm, ss)
    nc.vector.tensor_scalar_max(nrm, nrm, EPS)
    invn = scr_pool.tile([128, 2], FP, name=f"invn_{sid}", tag="invn")
    nc.vector.reciprocal(invn, nrm)
    for bt in range(2):
        ch = scr_pool.tile([128, D], FP, name=f"chat_{sid}_{bt}",
                           tag=f"chat{bt}")
        nc.vector.tensor_scalar_mul(ch, cur[bt], invn[:, bt:bt + 1])
        chat.append(ch)
    # transpose chat -> chatT (16 PE transposes via identity); the copy out
    # of PSUM rounds to f32r
    for bt in range(2):
        for dc in range(8):
            trp = smallps.tile([128, 128], FP, name=f"trp_{sid}_{bt}_{dc}",
                               tag="sps")
            nc.tensor.transpose(trp, chat[bt][:, dc * 128:(dc + 1) * 128],
                                ident)
            nc.vector.tensor_copy(chatT[:, dc, bt * 128:(bt + 1) * 128], trp)

    # ---- PSUM accumulators for this step
    u_ps = [[pools["ups"].tile([128, 512], FP, name=f"u_{sid}_{bt}_{nt}",
                               tag=f"u{bt}{nt}")
             for nt in range(2)] for bt in range(2)]
    se_ps = [pools["seps"].tile([128, 16], FP, name=f"se_{sid}_{bt}",
                                tag=f"se{bt}") for bt in range(2)]

    # ---- sweep the memory shard
    for j in range(nchunk):
        mh = pools["mh"].tile([128, D], mm_dt, name=f"mh_{sid}_{j}", tag="mh")
        nc.sync.dma_start(out=mh, in_=memT[j * 128:(j + 1) * 128, :])
        mn = pools["mn"].tile([128, D + 16], mm_dt, name=f"mn_{sid}_{j}",
                              tag="mn")
        nc.sync.dma_start(out=mn, in_=memN[j * 128:(j + 1) * 128, :])

        sim_ps = smallps.tile([128, 256], FP, name=f"sim_{sid}_{j}",
                              tag="sps")
        for dc in range(8):
            nc.tensor.matmul(
                sim_ps,
                lhsT=mh[:, dc * 128:(dc + 1) * 128],
                rhs=chatT[:, dc, :],
                start=(dc == 0), stop=(dc == 7),
            )
        expT = pools["expp"].tile([128, 256], mm_dt, name=f"exp_{sid}_{j}",
                                  tag="exp")
        nc.scalar.activation(
            expT, sim_ps, AFT.Exp,
            bias=bia_sb[:, j:j + 1], scale=scl_sb[:, j:j + 1],
        )
        for bt in range(2):
            for nt in range(2):
                nc.tensor.matmul(
                    u_ps[bt][nt],
                    lhsT=expT[:, bt * 128:(bt + 1) * 128],
                    rhs=mn[:, nt * 512:(nt + 1) * 512],
                    start=(j == 0), stop=(j == nchunk - 1),
                )
            nc.tensor.matmul(
                se_ps[bt],
                lhsT=expT[:, bt * 128:(bt + 1) * 128],
                rhs=mn[:, D:D + 16],
                start=(j == 0), stop=(j == nchunk - 1),
            )

    # ---- AllReduce partial (U, S) across the 8 cores
    b_in = pools["dram"].tile([256, D + 1], FP, name=f"bin_{sid}", tag="bin")
    b_out = pools["dram"].tile([256, D + 1], FP, name=f"bout_{sid}",
                               tag="bout", addr_space="Shared")
    for bt in range(2):
        u_sb = scr_pool.tile([128, D + 1], FP, name=f"usb_{sid}_{bt}",
                             tag=f"usb{bt}")
        for nt in range(2):
            nc.vector.tensor_copy(u_sb[:, nt * 512:(nt + 1) * 512],
                                  u_ps[bt][nt])
        nc.vector.tensor_copy(u_sb[:, D:D + 1], se_ps[bt][:, 0:1])
        nc.sync.dma_start(out=b_in[bt * 128:(bt + 1) * 128, :], in_=u_sb)
    nc.gpsimd.collective_compute(
        "AllReduce",
        AOP.add,
        replica_groups=[list(range(NCORES))],
        ins=[b_in.opt()],
        outs=[b_out.opt()],
    )

    # ---- read back, normalize, convex update
    for bt in range(2):
        u_t = scr_pool.tile([128, D + 1], FP, name=f"ug_{sid}_{bt}",
                            tag=f"ug{bt}")
        nc.sync.dma_start(out=u_t, in_=b_out[bt * 128:(bt + 1) * 128, :])
        invse = scr_pool.tile([128, 1], FP, name=f"invse_{sid}_{bt}",
                              tag="invse")
        nc.vector.reciprocal(invse, u_t[:, D:D + 1])
        att = scr_pool.tile([128, D], FP, name=f"att_{sid}_{bt}",
                            tag=f"att{bt}")
        # att = (U * 1/S) * 0.8
        nc.vector.tensor_scalar(
            out=att, in0=u_t[:, 0:D],
            scalar1=invse, scalar2=0.8,
            op0=AOP.mult, op1=AOP.mult,
        )
        c02 = scr_pool.tile([128, D], FP, name=f"c02_{sid}_{bt}",
                            tag=f"c02{bt}")
        nc.scalar.mul(c02, cur[bt], 0.2)
        nc.vector.tensor_add(cur[bt], att, c02)


_NC_CACHE = {}


def make_nc(nchunk=NCHUNK, steps=STEPS, reps=1, mm_dt=None):
    if mm_dt is None:
        mm_dt = MM_DT
    key = (nchunk, steps, reps, mm_dt)
    if key not in _NC_CACHE:
        nc = bacc.Bacc(
            "TRN2",
            target_bir_lowering=False,
            debug=False,
            num_devices=NCORES,
        )
        _emit(nc, nchunk, steps, reps, mm_dt)
        nc.finalize()
        _NC_CACHE[key] = nc
    return _NC_CACHE[key]


def _get_nc():
    return make_nc()


def _np_mm_dtype(mm_dt=None):
    if mm_dt is None:
        mm_dt = MM_DT
    return mybir.dt.np(mm_dt)


def _prep_inputs(query_trace, memory_matrix, nchunk=NCHUNK, mm_dt=None):
    mdt = _np_mm_dtype(mm_dt)
    csp = nchunk * 128
    q = np.ascontiguousarray(np.asarray(query_trace, dtype=np.float32))
    mem = np.asarray(memory_matrix, dtype=np.float32)
    assert q.shape == (B, D)
    c = mem.shape[0]

    cpad = NCORES * csp
    mem_pad = np.zeros((cpad, D), dtype=np.float32)
    mem_pad[:c] = mem
    norms = np.maximum(np.linalg.norm(mem_pad, axis=-1), EPS)
    inv = (1.0 / norms).astype(np.float32)
    inv[c:] = 0.0
    biasv = np.zeros(cpad, dtype=np.float32)
    biasv[c:] = PAD_BIAS

    in_maps = []
    for k in range(NCORES):
        r0, r1 = k * csp, (k + 1) * csp
        shard = mem_pad[r0:r1]
        # memT[j*128+p, dc*128+cc] = shard[j*128+cc, dc*128+p]
        memT = np.ascontiguousarray(
            shard.reshape(nchunk, 128, 8, 128).transpose(0, 3, 2, 1)
            .astype(mdt)
        ).reshape(csp, D)
        memn = np.zeros((csp, D + 16), dtype=mdt)
        memn[:, :D] = shard.astype(mdt)
        memn[:, D] = (np.arange(r0, r1) < c).astype(mdt)
        scl = np.ascontiguousarray(inv[r0:r1].reshape(nchunk, 128).T)
        bia = np.ascontiguousarray(biasv[r0:r1].reshape(nchunk, 128).T)
        in_maps.append({
            "memT": memT,
            "memN": memn,
            "scl": scl,
            "bia": bia,
            "qry": q,
        })
    return in_maps


def _run(in_maps, trace=False, **kwargs):
    nc = _get_nc()
    return bass_utils.run_bass_kernel_spmd(
        nc, in_maps, core_ids=list(range(NCORES)), trace=trace, **kwargs
    )


def kernel(query_trace, memory_matrix):
    in_maps = _prep_inputs(query_trace, memory_matrix)
    res = _run(in_maps)
    return np.asarray(res.results[0]["out"], dtype=np.float32)
